# revision 1
# baseline (speedup 1.0000x reference)
"""Distributed kNN OOD-score kernel for 8 Trainium2 NeuronCores.

Problem: for each of 4*32*32 query vectors (D=768), find the 3 nearest
database vectors (N=20000, squared-L2), average the 3 distances, and
bilinearly upsample the resulting [4,32,32] map to [4,1,512,512].

Sharding: queries are data-parallel. Each core owns half of one batch
image (16 of 32 query rows = 512 queries); the database is replicated
and streamed through SBUF in bf16. The one halo row each core needs for
the 16x bilinear upsample is exchanged with its pair core via a tiny
AllGather. Each core computes the 4-row block its PAIR needs first
(local tile 0), so the AllGather launches ~40us before the matmul
stream ends and its ~15us latency is fully hidden. The per-core
interpolation matrix (host input) absorbs the resulting row permutation,
keeping the device program SPMD-uniform.

Per-core device program:
  - scores t[q,n] = q.x - ||x||^2/2 via TensorE: 6 bf16 K=128 matmuls
    (stationary query tile, moving db columns) + one K=2 matmul adding
    the -||x||^2/2 row in split-bf16 (hi+lo) precision, accumulated f32
    in PSUM.
  - ScalarE evacuates each 500-col PSUM bank into an SBUF score strip.
  - VectorE max8 per [128,4000] strip -> per-strip top-8; final max8
    over the 40 strip winners -> global top-3 per query (values only).
  - mean distance = reduce_sum of sqrt((q^2 - 2t)/9) (ScalarE fused
    scale+bias+sqrt).
  - pair AllGather of local tile 0's 128 ood values (boundary block).
  - 16x bilinear upsample = two small f32 matmuls with interpolation
    matrices (built on host; verified against jax.image.resize).
"""

import sys

if "/opt/trn_rl_repo" not in sys.path:
    sys.path.insert(0, "/opt/trn_rl_repo")

import numpy as np
import ml_dtypes

import concourse.bass as bass
import concourse.bacc as bacc
import concourse.mybir as mybir
import concourse.tile as tile
from concourse import bass_utils

# Problem shape (hardcoded per contract).
B, D, H, W = 4, 768, 32, 32
N = 20000
K_NN = 3
OUT_H = OUT_W = 512
N_CORES = 8

SC = 4000           # db columns per super-chunk (8 PSUM banks * 500)
N_SC = N // SC      # 5
BANK = 500
N_BANK = SC // BANK  # 8
HALF = SC // 2      # db DMA granularity (finer for startup overlap)
KC = D // 128       # 6 contraction chunks
QPC = 512           # queries scored per core (16 rows)
N_QT = QPC // 128   # 4
OROWS = 256         # output rows per core
NCOL = 24           # ood columns entering the upsample (16 own + 2x4 gathered)

F32 = mybir.dt.float32
BF16 = mybir.dt.bfloat16
AX = mybir.AxisListType
AF = mybir.ActivationFunctionType

# local tile -> 4-row block of this core's half (block i = rows 4i..4i+3).
# Tile 0 is the block the PAIR core needs as its halo row: for the top
# half (rows 0-15) that's block 3 (row 15), for the bottom half (rows
# 16-31) block 0 (row 16).
TILE_BLOCKS = ([3, 0, 1, 2], [0, 1, 2, 3])


def _build_program():
    nc = bacc.Bacc(
        "TRN2", target_bir_lowering=False, debug=False, num_devices=N_CORES
    )
    dbT = nc.dram_tensor("dbT", [D, N], BF16, kind="ExternalInput").ap()
    xh = nc.dram_tensor("xh", [2, N], BF16, kind="ExternalInput").ap()
    qT = nc.dram_tensor("qT", [128, KC * QPC], BF16, kind="ExternalInput").ap()
    q2 = nc.dram_tensor("q2", [128, N_QT], F32, kind="ExternalInput").ap()
    art = nc.dram_tensor("art", [NCOL, OROWS], F32, kind="ExternalInput").ap()
    ac = nc.dram_tensor("ac", [W, OUT_W], F32, kind="ExternalInput").ap()
    out = nc.dram_tensor("out", [OROWS, OUT_W], F32, kind="ExternalOutput").ap()

    with tile.TileContext(nc) as tc:
        with (
            tc.tile_pool(name="static", bufs=1) as sp,
            tc.tile_pool(name="db", bufs=4 * KC) as dbp,
            tc.tile_pool(name="scores", bufs=2) as scp,
            tc.tile_pool(name="small", bufs=4) as smp,
            tc.tile_pool(name="psum", bufs=N_BANK, space="PSUM") as pp,
            tc.tile_pool(name="dram", bufs=1, space="DRAM") as dp,
        ):
            # queries per k-chunk so the first matmul only waits for k=0;
            # first super-chunk's db tiles loaded before everything else
            qk_sb = [
                sp.tile([128, QPC], BF16, name=f"qk{k}") for k in range(KC)
            ]
            db0_tiles = []
            for k in range(KC):
                nc.sync.dma_start(qk_sb[k][:], qT[:, k * QPC : (k + 1) * QPC])
                for h in range(2):
                    t = dbp.tile([128, HALF], BF16, tag="db", name=f"db0_{k}_{h}")
                    nc.sync.dma_start(
                        t[:], dbT[k * 128 : (k + 1) * 128, h * HALF : (h + 1) * HALF]
                    )
                    db0_tiles.append(t)
            xh_sb = sp.tile([2, N], BF16)
            nc.sync.dma_start(xh_sb[:], xh[:])
            q2_sb = sp.tile([128, N_QT], F32)
            nc.sync.dma_start(q2_sb[:], q2[:])
            art_sb = sp.tile([NCOL, OROWS], F32)
            nc.sync.dma_start(art_sb[:], art[:])
            ac_sb = sp.tile([W, OUT_W], F32)
            nc.sync.dma_start(ac_sb[:], ac[:])
            ones2 = sp.tile([2, 128], BF16)
            nc.gpsimd.memset(ones2[:], 1.0)

            # per-query-tile top-8 winners of each (super-chunk, bank)
            parts = [
                sp.tile([128, N_SC * N_BANK * 8], F32, name=f"part{qt}")
                for qt in range(N_QT)
            ]
            # qt 0 separate so the collective only depends on it
            oods = [
                sp.tile([128, 1], F32, name=f"ood{qt}") for qt in range(N_QT)
            ]
            cc_in = dp.tile([128], F32)
            cc_out = dp.tile([256], F32)
            scratch = dp.tile([QPC], F32)

            for sc in range(N_SC):
                if sc == 0:
                    db_tiles = db0_tiles
                else:
                    db_tiles = []
                    for k in range(KC):
                        for h in range(2):
                            t = dbp.tile(
                                [128, HALF], BF16, tag="db", name=f"db{k}_{h}"
                            )
                            nc.sync.dma_start(
                                t[:],
                                dbT[
                                    k * 128 : (k + 1) * 128,
                                    sc * SC + h * HALF : sc * SC + (h + 1) * HALF,
                                ],
                            )
                            db_tiles.append(t)
                for qt in range(N_QT):
                    bank_sb = [
                        scp.tile([128, BANK], F32, tag=f"sb{b}", name=f"sb{b}")
                        for b in range(N_BANK)
                    ]
                    banks = [
                        pp.tile([128, BANK], F32, tag="bank", name=f"bank{b}")
                        for b in range(N_BANK)
                    ]
                    for k in range(KC):
                        lhsT = qk_sb[k][:, qt * 128 : (qt + 1) * 128]
                        for b in range(N_BANK):
                            src = db_tiles[2 * k + (b * BANK) // HALF]
                            off = (b * BANK) % HALF
                            nc.tensor.matmul(
                                banks[b][:],
                                lhsT,
                                src[:, off : off + BANK],
                                start=(k == 0),
                                stop=False,
                            )
                    for b in range(N_BANK):
                        nc.tensor.matmul(
                            banks[b][:],
                            ones2[:],
                            xh_sb[:, sc * SC + b * BANK : sc * SC + (b + 1) * BANK],
                            start=False,
                            stop=True,
                        )
                    for b in range(N_BANK):
                        nc.scalar.activation(bank_sb[b][:], banks[b][:], AF.Copy)
                        nc.vector.max(
                            parts[qt][
                                :, (sc * N_BANK + b) * 8 : (sc * N_BANK + b + 1) * 8
                            ],
                            bank_sb[b][:],
                        )

                    if sc != N_SC - 1:
                        continue
                    # epilogue inline after this qt's last strip (engines run
                    # their queues in order — emitting it later would trap it
                    # behind the remaining strip maxes)
                    f8 = smp.tile([128, 8], F32, tag="f8")
                    nc.vector.max(f8[:], parts[qt][:])
                    # dist_j/3 = sqrt((q2 - 2 t_j) / 9); host passes q2/9
                    d3 = smp.tile([128, K_NN], F32, tag="d3")
                    nc.scalar.activation(
                        d3[:],
                        f8[:, 0:K_NN],
                        AF.Sqrt,
                        bias=q2_sb[:, qt : qt + 1],
                        scale=-2.0 / 9.0,
                    )
                    nc.vector.reduce_sum(oods[qt][:], d3[:], axis=AX.X)
                    if qt == 0:
                        # boundary block: gather it across the pair ASAP so
                        # the ~15us collective hides under remaining matmuls
                        nc.sync.dma_start(cc_in[:], oods[0][:])
                        nc.gpsimd.collective_compute(
                            "AllGather",
                            mybir.AluOpType.bypass,
                            replica_groups=[[0, 1], [2, 3], [4, 5], [6, 7]],
                            ins=[cc_in.opt()],
                            outs=[cc_out.opt()],
                        )
                    # own ood values -> scratch incrementally (local order)
                    nc.sync.dma_start(
                        scratch.rearrange("(q p) -> p q", p=128)[:, qt : qt + 1],
                        oods[qt][:],
                    )

            # ood_hT[c, j]: j 0..15 own rows (local order), 16..23 the two
            # gathered boundary blocks in rank order
            ood_hT = sp.tile([W, NCOL], F32)
            nc.sync.dma_start(
                ood_hT[:, 0:16], scratch.rearrange("(r c) -> c r", c=W)
            )
            nc.sync.dma_start(
                ood_hT[:, 16:NCOL],
                cc_out.rearrange("(b r c) -> c (b r)", b=2, c=W),
            )

            # P1[j, ow] = sum_c ood_hT[c, j] * A_c[c, ow]
            p1 = pp.tile([NCOL, OUT_W], F32, tag="bank")
            nc.tensor.matmul(p1[:], ood_hT[:], ac_sb[:], start=True, stop=True)
            p1_sb = sp.tile([NCOL, OUT_W], F32)
            nc.scalar.activation(p1_sb[:], p1[:], AF.Copy)
            # out[oi, ow] = sum_j art[j, oi] * P1[j, ow]
            for m in range(2):
                p2 = pp.tile([128, OUT_W], F32, tag="bank", name=f"p2_{m}")
                nc.tensor.matmul(
                    p2[:],
                    art_sb[:, m * 128 : (m + 1) * 128],
                    p1_sb[:],
                    start=True,
                    stop=True,
                )
                o_sb = smp.tile([128, OUT_W], F32, tag="osb", name=f"osb{m}")
                nc.scalar.activation(o_sb[:], p2[:], AF.Copy)
                nc.sync.dma_start(out[m * 128 : (m + 1) * 128, :], o_sb[:])

    nc.compile()
    return nc


def _bilinear_matrix(out_size: int, in_size: int) -> np.ndarray:
    """Half-pixel (align_corners=False) bilinear interpolation matrix
    [out_size, in_size]; edge-clamped, equivalent to jax.image.resize
    'bilinear' for integer upsampling."""
    A = np.zeros((out_size, in_size), dtype=np.float64)
    scale = in_size / out_size
    for i in range(out_size):
        s = (i + 0.5) * scale - 0.5
        j0 = int(np.floor(s))
        w = s - j0
        A[i, min(max(j0, 0), in_size - 1)] += 1.0 - w
        A[i, min(max(j0 + 1, 0), in_size - 1)] += w
    return A.astype(np.float32)


_NC_CACHE = None


def _get_nc():
    global _NC_CACHE
    if _NC_CACHE is None:
        _NC_CACHE = _build_program()
    return _NC_CACHE


def make_in_maps(embeddings: np.ndarray, database: np.ndarray):
    embeddings = np.asarray(embeddings, dtype=np.float32)
    database = np.asarray(database, dtype=np.float32)

    dbT = np.ascontiguousarray(database.T).astype(ml_dtypes.bfloat16)
    # -||x||^2/2 in split bf16 (hi + lo)
    xh_f = -0.5 * np.einsum("nd,nd->n", database, database)
    hi = xh_f.astype(ml_dtypes.bfloat16)
    lo = (xh_f - hi.astype(np.float32)).astype(ml_dtypes.bfloat16)
    xh = np.stack([hi, lo])

    q_all = embeddings.transpose(0, 2, 3, 1).reshape(B, H * W, D)
    Ac = _bilinear_matrix(OUT_W, W)                      # [512, 32]
    Ar = _bilinear_matrix(OUT_H, H)                      # [512, 32]
    # the two gathered blocks in cc_out rank order: pair-core tile 0 rows
    cc_rows = [12, 13, 14, 15, 16, 17, 18, 19]

    in_maps = []
    for c in range(N_CORES):
        b, half = divmod(c, 2)
        blocks = TILE_BLOCKS[half]
        own_rows = [16 * half + 4 * blk + r for blk in blocks for r in range(4)]

        # queries in local-tile order
        q = np.concatenate(
            [
                q_all[b, (16 * half + 4 * blk) * W : (16 * half + 4 * blk + 4) * W]
                for blk in blocks
            ]
        )                                                # [512, 768]
        qTb = (
            np.ascontiguousarray(q.T)                    # [768, 512]
            .reshape(KC, 128, QPC)
            .transpose(1, 0, 2)
            .reshape(128, KC * QPC)
            .astype(ml_dtypes.bfloat16)
        )
        q2 = np.einsum("qd,qd->q", q, q) / 9.0
        q2 = np.ascontiguousarray(q2.reshape(N_QT, 128).T.astype(np.float32))

        # interpolation rows matching ood_hT's column order
        Arh = Ar[half * OROWS : (half + 1) * OROWS]      # [256, 32]
        art = np.zeros((NCOL, OROWS), dtype=np.float32)
        for j, row in enumerate(own_rows):
            art[j] = Arh[:, row]
        for j, row in enumerate(cc_rows):
            if row not in own_rows:
                art[16 + j] = Arh[:, row]
        in_maps.append(
            {
                "dbT": dbT,
                "xh": xh,
                "qT": qTb,
                "q2": q2,
                "art": art,
                "ac": np.ascontiguousarray(Ac.T),        # [32, 512]
            }
        )
    return in_maps


def run_device(in_maps, **kwargs):
    nc = _get_nc()
    return bass_utils.run_bass_kernel_spmd(
        nc, in_maps, core_ids=list(range(N_CORES)), **kwargs
    )


def kernel(embeddings, database, k, out_h, out_w):
    assert int(k) == K_NN and int(out_h) == OUT_H and int(out_w) == OUT_W
    in_maps = make_in_maps(np.asarray(embeddings), np.asarray(database))
    res = run_device(in_maps)
    out = np.empty((B, 1, OUT_H, OUT_W), dtype=np.float32)
    for c in range(N_CORES):
        b, half = divmod(c, 2)
        out[b, 0, half * OROWS : (half + 1) * OROWS] = res.results[c]["out"]
    return out



# revision 3
# speedup vs baseline: 2.7560x; 2.7560x over previous
"""Distributed kNN OOD-score kernel for 8 Trainium2 NeuronCores.

Problem: for each of 4*32*32 query vectors (D=768), find the 3 nearest
database vectors (N=20000, squared-L2), average the 3 distances, and
bilinearly upsample the resulting [4,32,32] map to [4,1,512,512].

Sharding: queries are data-parallel. Each core owns half of one batch
image (16 of 32 query rows = 512 queries); the database is replicated
and streamed through SBUF in fp8 (e4m3). The one halo row each core
needs for the 16x bilinear upsample is exchanged with its pair core via
a tiny AllGather whose ~15us fixed latency is hidden by hoisting the
boundary query tile (qt0) of the last two super-chunks ahead of the
other tiles (the fp8 database tiles stay resident long enough to allow
the out-of-order walk).

Per-core device program, per (super-chunk sc, query-tile qt):
  - scores t[q,n] = q.x - (||x||^2 - 768)/2 via TensorE fp8 DoubleRow
    matmuls: 3 chunks of K=256 at 0.5 cycles/row (4x bf16 rate) + one
    K=4 DoubleRow adding the centered -||x||^2/2 row in 4-way split-fp8
    precision, accumulated f32 in PSUM (8 banks, bank-major fill).
  - ScalarE evacuates PSUM banks 0-5 (two [128,3,500] copies) into a
    bf16 strip; VectorE max-fuses banks 6-7 against strip[0:1000]
    (TensorTensor may read one PSUM operand), then a bf16 pairwise-max
    ladder (2x DVE mode) reduces 4000 -> 250 candidates; per sc-pair one
    max8 keeps the top-8 (exact top-3 up to ~0.1%/query collisions in
    the depth-16 max groups, well inside tolerance).
  - final max8 over 24 strip winners -> top-3 scores; mean distance =
    reduce_sum of sqrt((q^2+768-2t)/9) (ScalarE fused scale+bias+sqrt).
  - 16x bilinear upsample = two small bf16 matmuls with interpolation
    matrices (host-built; weights are dyadic so bf16-exact).
"""

import sys

if "/opt/trn_rl_repo" not in sys.path:
    sys.path.insert(0, "/opt/trn_rl_repo")

import numpy as np
import ml_dtypes

import concourse.bass as bass
import concourse.bacc as bacc
import concourse.mybir as mybir
import concourse.tile as tile
from concourse import bass_utils

# Problem shape (hardcoded per contract).
B, D, H, W = 4, 768, 32, 32
N = 20000
K_NN = 3
OUT_H = OUT_W = 512
N_CORES = 8

SC = 4000            # db columns per super-chunk
N_SC = N // SC       # 5
BANK = 500           # db columns per PSUM bank
CH = D // 256        # 3 DoubleRow contraction chunks (K=256 each)
QPC = 512            # queries scored per core (16 rows)
N_QT = QPC // 128    # 4
OROWS = 256          # output rows per core
NCOL = 24            # ood columns entering the upsample (16 own + 2x4 gathered)

F32 = mybir.dt.float32
BF16 = mybir.dt.bfloat16
FP8 = mybir.dt.float8e4
AX = mybir.AxisListType
AF = mybir.ActivationFunctionType
DR = mybir.MatmulPerfMode.DoubleRow

# local tile -> 4-row block of this core's half (block i = rows 4i..4i+3).
# Tile 0 is the block the PAIR core needs as its halo row: for the top
# half (rows 0-15) that's block 3 (row 15), for the bottom half (rows
# 16-31) block 0 (row 16).
TILE_BLOCKS = ([3, 0, 1, 2], [0, 1, 2, 3])

# qt0 of the last two super-chunks is hoisted so its ood (the boundary
# block) finishes ~20us before the instruction stream ends, hiding the
# AllGather's fixed latency.
PHASES = [
    (0, (0, 1, 2, 3)),
    (1, (0, 1, 2, 3)),
    (2, (0, 1, 2, 3)),
    (3, (0,)),
    (4, (0,)),
    (3, (1, 2, 3)),
    (4, (1, 2, 3)),
]


def _build_program():
    nc = bacc.Bacc(
        "TRN2", target_bir_lowering=False, debug=False, num_devices=N_CORES
    )
    dbd = nc.dram_tensor("dbd", [128, N_SC, CH, 2, SC], FP8, kind="ExternalInput").ap()
    qd = nc.dram_tensor("qd", [128, N_QT, CH, 2, 128], FP8, kind="ExternalInput").ap()
    xhd = nc.dram_tensor("xhd", [2, 2, N], FP8, kind="ExternalInput").ap()
    q2 = nc.dram_tensor("q2", [128, N_QT], F32, kind="ExternalInput").ap()
    art = nc.dram_tensor("art", [NCOL, OROWS], BF16, kind="ExternalInput").ap()
    ac = nc.dram_tensor("ac", [W, OUT_W], BF16, kind="ExternalInput").ap()
    out = nc.dram_tensor("out", [OROWS, OUT_W], F32, kind="ExternalOutput").ap()

    with tile.TileContext(nc) as tc:
        with (
            tc.tile_pool(name="static", bufs=1) as sp,
            tc.tile_pool(name="db", bufs=3) as dbp,
            tc.tile_pool(name="strip", bufs=2) as stp,
            tc.tile_pool(name="lad", bufs=2) as ltp,
            tc.tile_pool(name="small", bufs=4) as smp,
            tc.tile_pool(name="psum", bufs=1, space="PSUM") as pp,
            tc.tile_pool(name="dram", bufs=1, space="DRAM") as dp,
        ):
            qk = sp.tile([128, N_QT, CH, 2, 128], FP8, name="qk")
            nc.sync.dma_start(qk[:], qd[:])
            xh4 = sp.tile([2, 2, N], FP8, name="xh4")
            nc.sync.dma_start(xh4[:], xhd[:])
            q2_sb = sp.tile([128, N_QT], F32, name="q2_sb")
            nc.sync.dma_start(q2_sb[:], q2[:])
            art_sb = sp.tile([NCOL, OROWS], BF16, name="art_sb")
            nc.sync.dma_start(art_sb[:], art[:])
            ac_sb = sp.tile([W, OUT_W], BF16, name="ac_sb")
            nc.sync.dma_start(ac_sb[:], ac[:])
            ones4 = sp.tile([2, 2, 128], FP8, name="ones4")
            nc.gpsimd.memset(ones4[:], 1.0)

            # db tiles: 3-buffer rotation; sc3/sc4 reuse sc0/sc1's space
            # (their DMAs wait for the last phase-0/1 matmul reads), and
            # both stay live through the trailing qt1-3 phases.
            db_t = {}
            for sc in range(N_SC):
                t = dbp.tile([128, CH, 2, SC], FP8, tag="db", name=f"db{sc}")
                db_t[sc] = t
                nparts = 4 if sc == 0 else 2
                step = SC // nparts
                for h in range(nparts):
                    nc.sync.dma_start(
                        t[:, :, :, h * step : (h + 1) * step],
                        dbd[:, sc, :, :, h * step : (h + 1) * step],
                    )

            # PSUM: banks 0-2 / 3-5 (ScalarE evac) and 6-7 (DVE fused max)
            P0 = pp.tile([128, 3, 512], F32, tag="pA", name="P0")
            P1 = pp.tile([128, 3, 512], F32, tag="pB", name="P1")
            P2 = pp.tile([128, 2, 512], F32, tag="pC", name="P2")

            Dbuf = sp.tile([128, N_QT, N_SC, 250], BF16, name="Dbuf")
            parts = sp.tile([128, N_QT, 24], BF16, name="parts")
            oods = [
                sp.tile([128, 1], BF16, name=f"ood{qt}") for qt in range(N_QT)
            ]
            cc_in = dp.tile([128], BF16, name="cc_in")
            cc_out = dp.tile([256], BF16, name="cc_out")
            scratch = dp.tile([QPC], BF16, name="scratch")

            def emit_scqt(sc, qt):
                # bank-major fill so PSUM groups complete in evac order
                for b in range(8):
                    pi, sub = (0, b) if b < 3 else ((1, b - 3) if b < 6 else (2, b - 6))
                    o = (P0, P1, P2)[pi][:, sub, 0:BANK]
                    for ch in range(CH):
                        nc.tensor.matmul(
                            o,
                            qk[:, qt, ch],
                            db_t[sc][:, ch, :, BANK * b : BANK * (b + 1)],
                            start=(ch == 0),
                            stop=False,
                            perf_mode=DR,
                        )
                    nc.tensor.matmul(
                        o,
                        ones4[:],
                        xh4[:, :, sc * SC + BANK * b : sc * SC + BANK * (b + 1)],
                        start=False,
                        stop=True,
                        perf_mode=DR,
                    )
                strip = stp.tile([128, 3000], BF16, tag="strip", name="strip")
                sv = strip.rearrange("p (a b n) -> p a b n", a=2, b=3)
                nc.scalar.activation(sv[:, 0], P0[:, :, 0:BANK], AF.Copy)
                nc.scalar.activation(sv[:, 1], P1[:, :, 0:BANK], AF.Copy)
                # pairwise-max ladder: 4000 -> 250 exact-value candidates
                Ht = ltp.tile([128, 1000], BF16, tag="H", name="Ht")
                nc.vector.tensor_max(Ht[:], P2[:, :, 0:BANK], strip[:, 0:1000])
                At = ltp.tile([128, 1000], BF16, tag="A", name="At")
                nc.vector.tensor_max(At[:], strip[:, 1000:2000], strip[:, 2000:3000])
                Bt = ltp.tile([128, 1000], BF16, tag="Bv", name="Bt")
                nc.vector.tensor_max(Bt[:], At[:], Ht[:])
                Ct = ltp.tile([128, 500], BF16, tag="C", name="Ct")
                nc.vector.tensor_max(Ct[:], Bt[:, 0:500], Bt[:, 500:1000])
                nc.vector.tensor_max(Dbuf[:, qt, sc], Ct[:, 0:250], Ct[:, 250:500])

            def emit_pair_max8(qt, pair_idx):
                i0 = pair_idx * 8
                if pair_idx < 2:
                    src = Dbuf[:, qt, 2 * pair_idx : 2 * pair_idx + 2]
                else:
                    src = Dbuf[:, qt, 4]
                nc.vector.max(parts[:, qt, i0 : i0 + 8], src)

            def emit_qt_final(qt):
                f8 = smp.tile([128, 8], BF16, tag="f8", name="f8")
                nc.vector.max(f8[:], parts[:, qt])
                # dist_j/3 = sqrt((q2 + 768 - 2 t_j) / 9); host passes
                # (q2+768)/9 as the bias
                d3 = smp.tile([128, K_NN], BF16, tag="d3", name="d3")
                nc.scalar.activation(
                    d3[:],
                    f8[:, 0:K_NN],
                    AF.Sqrt,
                    bias=q2_sb[:, qt : qt + 1],
                    scale=-2.0 / 9.0,
                )
                with nc.allow_low_precision(reason="3-term mean; bf16 ample"):
                    nc.vector.reduce_sum(oods[qt][:], d3[:], axis=AX.X)
                nc.sync.dma_start(
                    scratch.rearrange("(q p) -> p q", p=128)[:, qt : qt + 1],
                    oods[qt][:],
                )
                if qt == 0:
                    # boundary block: gather across the pair while qt1-3
                    # of sc3/sc4 still run (~20us of cover)
                    nc.sync.dma_start(cc_in[:], oods[0][:])
                    nc.gpsimd.collective_compute(
                        "AllGather",
                        mybir.AluOpType.bypass,
                        replica_groups=[[0, 1], [2, 3], [4, 5], [6, 7]],
                        ins=[cc_in.opt()],
                        outs=[cc_out.opt()],
                    )

            for sc, qts in PHASES:
                for qt in qts:
                    emit_scqt(sc, qt)
                    if sc == 1:
                        emit_pair_max8(qt, 0)
                    elif sc == 3:
                        emit_pair_max8(qt, 1)
                    elif sc == 4:
                        emit_pair_max8(qt, 2)
                        emit_qt_final(qt)

            # ood_hT[c, j]: j 0..15 own rows (local order), 16..23 the two
            # gathered boundary blocks in rank order
            ood_hT = sp.tile([W, NCOL], BF16, name="ood_hT")
            nc.sync.dma_start(
                ood_hT[:, 0:16], scratch.rearrange("(r c) -> c r", c=W)
            )
            nc.sync.dma_start(
                ood_hT[:, 16:NCOL],
                cc_out.rearrange("(b r c) -> c (b r)", b=2, c=W),
            )

            # P1[j, ow] = sum_c ood_hT[c, j] * A_c[c, ow]
            p1p = pp.tile([NCOL, OUT_W], F32, tag="pC", name="p1p")
            nc.tensor.matmul(p1p[:], ood_hT[:], ac_sb[:], start=True, stop=True)
            p1_sb = sp.tile([NCOL, OUT_W], BF16, name="p1_sb")
            nc.scalar.activation(p1_sb[:], p1p[:], AF.Copy)
            # out[oi, ow] = sum_j art[j, oi] * P1[j, ow]
            for m in range(2):
                p2p = pp.tile(
                    [128, OUT_W], F32, tag=("pA", "pB")[m], name=f"p2p{m}"
                )
                nc.tensor.matmul(
                    p2p[:],
                    art_sb[:, m * 128 : (m + 1) * 128],
                    p1_sb[:],
                    start=True,
                    stop=True,
                )
                o_sb = smp.tile([128, OUT_W], F32, tag="osb", name=f"osb{m}")
                nc.scalar.activation(o_sb[:], p2p[:], AF.Copy)
                nc.sync.dma_start(out[m * 128 : (m + 1) * 128, :], o_sb[:])

    nc.compile()
    return nc


def _bilinear_matrix(out_size: int, in_size: int) -> np.ndarray:
    """Half-pixel (align_corners=False) bilinear interpolation matrix
    [out_size, in_size]; edge-clamped, equivalent to jax.image.resize
    'bilinear' for integer upsampling."""
    A = np.zeros((out_size, in_size), dtype=np.float64)
    scale = in_size / out_size
    for i in range(out_size):
        s = (i + 0.5) * scale - 0.5
        j0 = int(np.floor(s))
        w = s - j0
        A[i, min(max(j0, 0), in_size - 1)] += 1.0 - w
        A[i, min(max(j0 + 1, 0), in_size - 1)] += w
    return A.astype(np.float32)


_NC_CACHE = None


def _get_nc():
    global _NC_CACHE
    if _NC_CACHE is None:
        _NC_CACHE = _build_program()
    return _NC_CACHE


def _fp8_split4(v: np.ndarray) -> np.ndarray:
    """4-way residual split of f32 vector v into fp8 e4m3 rows that sum
    (in f32) back to v to ~1e-4 absolute."""
    rows = []
    r = v.astype(np.float64)
    for _ in range(4):
        s = np.asarray(r, dtype=np.float32).astype(ml_dtypes.float8_e4m3)
        rows.append(s)
        r = r - s.astype(np.float64)
    return np.stack(rows)  # [4, N]


def make_in_maps(embeddings: np.ndarray, database: np.ndarray):
    embeddings = np.asarray(embeddings, dtype=np.float32)
    database = np.asarray(database, dtype=np.float32)

    x8 = database.astype(ml_dtypes.float8_e4m3)          # [N, D]
    # dbd[p, sc, ch, i, n] = x8[sc*SC+n, 256ch+128i+p]
    dbT = np.ascontiguousarray(x8.T)                     # [D, N]
    dbd = np.ascontiguousarray(
        dbT.reshape(CH, 2, 128, N_SC, SC).transpose(2, 3, 0, 1, 4)
    )
    # centered score shift s = (768 - ||x8||^2)/2, 4-way split fp8
    xs = x8.astype(np.float32)
    s = (D - np.einsum("nd,nd->n", xs, xs)) * 0.5
    sp4 = _fp8_split4(s)                                 # [4, N]
    xhd = np.ascontiguousarray(sp4.reshape(2, 2, N).transpose(1, 0, 2))

    q_all = embeddings.transpose(0, 2, 3, 1).reshape(B, H * W, D)
    Ac = _bilinear_matrix(OUT_W, W)                      # [512, 32]
    Ar = _bilinear_matrix(OUT_H, H)                      # [512, 32]
    # the two gathered blocks in cc_out rank order: pair-core tile 0 rows
    cc_rows = [12, 13, 14, 15, 16, 17, 18, 19]

    in_maps = []
    for c in range(N_CORES):
        b, half = divmod(c, 2)
        blocks = TILE_BLOCKS[half]
        own_rows = [16 * half + 4 * blk + r for blk in blocks for r in range(4)]

        # queries in local-tile order
        q = np.concatenate(
            [
                q_all[b, (16 * half + 4 * blk) * W : (16 * half + 4 * blk + 4) * W]
                for blk in blocks
            ]
        )                                                # [512, 768]
        q8 = q.astype(ml_dtypes.float8_e4m3)
        # qd[p, qt, ch, i, m] = q8[128qt+m, 256ch+128i+p]
        qT = np.ascontiguousarray(q8.T)                  # [768, 512]
        qdc = np.ascontiguousarray(
            qT.reshape(CH, 2, 128, N_QT, 128).transpose(2, 3, 0, 1, 4)
        )
        q8f = q8.astype(np.float32)
        q2v = (np.einsum("qd,qd->q", q8f, q8f) + D) / 9.0
        q2v = np.ascontiguousarray(
            q2v.reshape(N_QT, 128).T.astype(np.float32)
        )

        # interpolation rows matching ood_hT's column order
        Arh = Ar[half * OROWS : (half + 1) * OROWS]      # [256, 32]
        art = np.zeros((NCOL, OROWS), dtype=np.float32)
        for j, row in enumerate(own_rows):
            art[j] = Arh[:, row]
        for j, row in enumerate(cc_rows):
            if row not in own_rows:
                art[16 + j] = Arh[:, row]
        in_maps.append(
            {
                "dbd": dbd,
                "qd": qdc,
                "xhd": xhd,
                "q2": q2v,
                "art": art.astype(ml_dtypes.bfloat16),
                "ac": np.ascontiguousarray(Ac.T).astype(ml_dtypes.bfloat16),
            }
        )
    return in_maps


def run_device(in_maps, **kwargs):
    nc = _get_nc()
    return bass_utils.run_bass_kernel_spmd(
        nc, in_maps, core_ids=list(range(N_CORES)), **kwargs
    )


def kernel(embeddings, database, k, out_h, out_w):
    assert int(k) == K_NN and int(out_h) == OUT_H and int(out_w) == OUT_W
    in_maps = make_in_maps(np.asarray(embeddings), np.asarray(database))
    res = run_device(in_maps)
    out = np.empty((B, 1, OUT_H, OUT_W), dtype=np.float32)
    for c in range(N_CORES):
        b, half = divmod(c, 2)
        out[b, 0, half * OROWS : (half + 1) * OROWS] = res.results[c]["out"]
    return out
